# revision 23
# baseline (speedup 1.0000x reference)
"""Trainium2 Bass kernel for nn_AdaptiveResBlock (8-core data-parallel).

Reference computation (per batch element b, C=256 channels, T=8192 time):
  for i, dil in enumerate((1, 2, 4)):
      xt = lrelu(x)
      xP, xF = time-gather of xt at round(t -/+ d*dil), zero out-of-range
      xt = WC@xt + WP@xP + WF@xF + biases        (1x1 convs over channels)
      xt = lrelu(xt)
      xt = conv3(xt, WA) + bias
      x = xt + x

Structure used:
  * The time-gather commutes with the 1x1 convs:
    WP @ gather(xt) == gather(WP @ xt).
  * B-stage: u^T = [WP;WF] @ xt computed token-major on the PE
    (xt-stationary matmuls), ACT-copied as bf16 st tiles [128 tok, 512].
  * Channel-major windowed one-hot gather: offsets are bounded by
    16*dilation, so every output token whose P-source lies in token
    block j sits in the (128+pad)-token window [128j, 128j+128+pad)
    with pad = 16*dil (F: [128j-pad, 128j+128)).  One matmul per
    (dir, c-half, src block):
      psum[c, t] += st[j][:, c-slice]^T @ S_dir[j][src, t-window]
    lands the gathered conv result DIRECTLY channel-major in PSUM, on top
    of the WC matmuls accumulating in the same bank -- no PE transposes,
    no separate gather pass.  S is built on DVE via is_equal of
    preloaded window-relative indices (int8, -128 sentinel) against an
    iota column; out-of-range indices match no block => free zero mask.
  * v = Prelu(psum) straight from PSUM (ACT); conv3 runs channel-major
    off v tiles with 1-column halos; residual update and the next
    iteration's lrelu are fused right behind each tile (DVE).
  * Weights are stored in DRAM pre-transposed to the exact SBUF layout
    (partition-major) so every weight DMA is a contiguous per-partition
    row; rel window-index tensors stream in 16-block chunks interleaved
    with compute, and iteration i+1's weights/rel prefetch during
    iteration i.

Sharded data-parallel over B=8 across the 8 NeuronCores; weights
replicated; per-core window-index tensors precomputed from d on host.
"""

import numpy as np
import ml_dtypes
from contextlib import ExitStack

import concourse.bass as bass
import concourse.tile as tile
from concourse import mybir, bacc
from concourse.bass_utils import run_bass_kernel_spmd

F32 = mybir.dt.float32
BF16 = mybir.dt.bfloat16
AF = mybir.ActivationFunctionType
OP = mybir.AluOpType

B, C, T_FULL = 8, 256, 8192
DILATIONS = (1, 2, 4)
PADS = tuple(16 * d for d in DILATIONS)   # max gather offset per iteration
NITER = len(DILATIONS)
SLOPE = 0.1
LAG = 8        # B-stage leads the consume stage by this many token blocks
RCHUNK = 8     # rel DMA chunk, in 128-token blocks


def _pieces(tt, nR, T, pad):
    """Gather matmul pieces for 512-token tile tt at max offset `pad`.
    Returns list of (j, rhs_a, rhs_b, ps_a) with rhs cols into the [128,384]
    S tile (P window at 0, F window at 192) and psum column offset."""
    L, R = 512 * tt, 512 * tt + 512
    out = []
    for j in range(max(0, 4 * tt - 1), min(nR, 4 * tt + 5)):
        # P window: t in [128j, 128j+128+pad)
        a = max(L, 128 * j, 0)
        b = min(R, 128 * j + 128 + pad, T)
        if a < b:
            out.append((j, a - 128 * j, b - 128 * j, a - L))
        # F window: t in [128j-pad, 128j+128); rel cols based at 128j-64
        a = max(L, 128 * j - pad, 0)
        b = min(R, 128 * j + 128, T)
        if a < b:
            w0 = 128 * j - 64
            out.append((j, 192 + a - w0, 192 + b - w0, a - L))
    return out


def build_nc(T=T_FULL, num_devices=8, has_b1=False):
    nT = T // 512            # 512-wide time tiles
    nR = T // 128            # 128-wide token blocks
    rchunk = RCHUNK if nR % RCHUNK == 0 else nR
    nchunks = nR // rchunk

    nc = bacc.Bacc("TRN2", target_bir_lowering=False, debug=False,
                   num_devices=num_devices)
    x_d = nc.declare_dram_parameter("x", [2, 128, T], BF16, isOutput=False)
    wpf_d = nc.declare_dram_parameter("wpf", [NITER, 128, 2, 512], BF16,
                                      isOutput=False)
    wcc_d = nc.declare_dram_parameter("wcc", [NITER, 128, 2, 2, 128], BF16,
                                      isOutput=False)
    wa_d = nc.declare_dram_parameter("wa", [NITER, 128, 3, 2, 2, 128], BF16,
                                     isOutput=False)
    b3_d = nc.declare_dram_parameter("b3", [NITER, 128, 2], F32,
                                     isOutput=False)
    rel_d = nc.declare_dram_parameter("rel", [NITER, 128, nR, 384],
                                      mybir.dt.int8, isOutput=False)
    iota_d = nc.declare_dram_parameter("iota", [128, 1], F32, isOutput=False)
    if has_b1:
        b1_d = nc.declare_dram_parameter("b1", [NITER, 128, 2], F32,
                                         isOutput=False)
    out_d = nc.declare_dram_parameter("out", [2, 128, T], F32, isOutput=True)

    with tile.TileContext(nc) as tc, ExitStack() as ctx:
        xpool = ctx.enter_context(tc.tile_pool(name="xres", bufs=1))
        stg = ctx.enter_context(tc.tile_pool(name="stg", bufs=4))
        stp = ctx.enter_context(tc.tile_pool(name="stp", bufs=13))
        sp = ctx.enter_context(tc.tile_pool(name="sp", bufs=13))
        relp = ctx.enter_context(tc.tile_pool(name="relp", bufs=2))
        xtp = ctx.enter_context(tc.tile_pool(name="xtp", bufs=16))
        vp = ctx.enter_context(tc.tile_pool(name="vp", bufs=4))
        wts = ctx.enter_context(tc.tile_pool(name="wts", bufs=2))
        cst = ctx.enter_context(tc.tile_pool(name="cst", bufs=1))
        ps_ps = ctx.enter_context(tc.tile_pool(name="ps", bufs=8, space="PSUM"))
        pu_ps = pw_ps = pc_ps = ps_ps

        def load_rel_blocks(rel_sb, i, b0, nblk):
            sl = bass.ds(b0, nblk)
            nc.scalar.dma_start(rel_sb[:, sl, :], rel_d[i][:, sl, :])

        def load_w(i, what):
            # contiguous per-partition rows: fast, small DMAs
            if what == "wpf":
                t = wts.tile([128, 2, 512], BF16, tag="wpf")
                nc.scalar.dma_start(t[:, :, :], wpf_d[i])
            elif what == "wcc":
                t = wts.tile([128, 2, 2, 128], BF16, tag="wcc")
                nc.scalar.dma_start(t[:, :, :, :], wcc_d[i])
            elif what == "wa":
                t = wts.tile([128, 3, 2, 2, 128], BF16, tag="wa")
                nc.scalar.dma_start(t[:, :, :, :, :], wa_d[i])
            elif what == "b3":
                t = wts.tile([128, 2], F32, tag="b3")
                nc.scalar.dma_start(t[:, :], b3_d[i])
            elif what == "b1":
                t = wts.tile([128, 2], F32, tag="b1")
                nc.scalar.dma_start(t[:, :], b1_d[i])
            return t

        # ---- startup: finest-granularity head of x so the first B matmul
        # can issue as soon as one 128-token block + wpf have landed ----
        # x streams in (and the residual accumulates) in bf16: every matmul
        # consumer is bf16 anyway, and it halves input HBM traffic; only
        # the final output stages through fp32 tiles for the DMA out.
        x_sb = xpool.tile([128, 2, T], BF16)
        for cb in range(2):
            nc.sync.dma_start(x_sb[:, cb, 0:128], x_d[cb, :, 0:128])
        for cb in range(2):
            nc.sync.dma_start(x_sb[:, cb, 128:512], x_d[cb, :, 128:512])
        # iteration-0 weights ride the scalar queue, wpf (B-stage) first
        w0 = {"wpf": load_w(0, "wpf")}
        iota_sb = cst.tile([128, 1], F32)
        nc.scalar.dma_start(iota_sb[:, :], iota_d[:, :])
        w0["wcc"] = load_w(0, "wcc")
        rel0 = relp.tile([128, nR, 384], mybir.dt.int8, tag="rel")
        load_rel_blocks(rel0, 0, 0, min(4, nR))
        w0["wa"] = load_w(0, "wa")
        w0["b3"] = load_w(0, "b3")
        if has_b1:
            w0["b1"] = load_w(0, "b1")
        # rest of x on the sync queue
        HEAD = min(2048, T)
        for sg in range((HEAD - 512) // 512):
            sl = bass.ds(512 + sg * 512, 512)
            for cb in range(2):
                nc.sync.dma_start(x_sb[:, cb, sl], x_d[cb, :, sl])
        for sg in range((T - HEAD) // 2048):
            sl = bass.ds(HEAD + sg * 2048, 2048)
            for cb in range(2):
                nc.sync.dma_start(x_sb[:, cb, sl], x_d[cb, :, sl])

        st_tiles = [None] * nR
        S_tiles = [None] * nR
        xt_tiles = [None] * nT
        xt_next = [None] * nT
        v_tiles = [None] * nT

        def emit_A(tt):
            # lrelu on the ACT engine (Prelu): frees the DVE and decouples
            # the x-stream critical path from the vector queue
            tsl = bass.ts(tt, 512)
            xt = xtp.tile([128, 2, 512], BF16, tag="xt")
            nc.scalar.activation(xt[:, :, :], x_sb[:, :, tsl],
                                 AF.Prelu, alpha=SLOPE)
            return xt

        def emit_A0_block(blk):
            # iteration-0 tile 0 is built per 128-token block to shorten
            # the DMA->lrelu->first-matmul critical path
            sl = bass.ds(blk * 128, 128)
            nc.scalar.activation(xt_tiles[0][:, :, sl], x_sb[:, :, sl],
                                 AF.Prelu, alpha=SLOPE)

        def emit_conv3(tt, wa_sb, b3_sb, last):
            tsl = bass.ts(tt, 512)
            for ob in range(2):
                py = pc_ps.tile([128, 512], F32, tag="ps")
                j = 0
                for k in range(3):
                    for cb in range(2):
                        nc.tensor.matmul(py[:, :], wa_sb[:, k, cb, ob, :],
                                         v_tiles[tt][:, cb, k:k + 512],
                                         start=(j == 0), stop=(j == 5))
                        j += 1
                if last:
                    # final iteration: residual lands in fp32 staging
                    # tiles (DMA cannot convert dtypes), in halves on the
                    # very last tile so the final out-DMA starts early
                    nh = 2 if tt == nT - 1 else 1
                    for h in range(nh):
                        w = 512 // nh
                        hsl = bass.ds(512 * tt + w * h, w)
                        sg = stg.tile([128, 512], F32, tag="stg")
                        nc.vector.scalar_tensor_tensor(
                            sg[:, 0:w], py[:, w * h:w * h + w],
                            b3_sb[:, ob:ob + 1], x_sb[:, ob, hsl],
                            OP.add, OP.add)
                        nc.sync.dma_start(out_d[ob, :, hsl], sg[:, 0:w])
                else:
                    nc.vector.scalar_tensor_tensor(
                        x_sb[:, ob, tsl], py[:, :], b3_sb[:, ob:ob + 1],
                        x_sb[:, ob, tsl], OP.add, OP.add)

        cur_w = cur_rel = None
        nxt_w = nxt_rel = None
        for i in range(NITER):
            if i == 0:
                cur_w, cur_rel = w0, rel0
            else:
                cur_w, cur_rel = nxt_w, nxt_rel
            nxt_w, nxt_rel = {}, None
            wpf_sb, wcc_sb, wa_sb = cur_w["wpf"], cur_w["wcc"], cur_w["wa"]
            b3_sb, rel_sb = cur_w["b3"], cur_rel
            b1_sb = cur_w.get("b1")
            pad = PADS[i]

            if i != 0:
                xt_tiles, xt_next = xt_next, [None] * nT

            # rel DMA schedule: keep rel traffic out of the early x-stream
            # bandwidth window; each chunk still lands well before its
            # S-builds are consumed LAG blocks later
            rel0_sched = {}
            nxt_sched = {}
            if nR > 4:
                rel0_sched[1] = (4, min(4, nR - 4))
            for k in range(1, (nR + rchunk - 1) // rchunk):
                rel0_sched[5 * k + 3] = (rchunk * k,
                                         min(rchunk, nR - rchunk * k))
            for k in range(0, (nR + rchunk - 1) // rchunk):
                nxt_sched[40 + 3 * k] = (rchunk * k,
                                         min(rchunk, nR - rchunk * k))

            def emit_B(b):
                tt, off = b // 4, (b % 4) * 128
                ps = pu_ps.tile([128, 512], F32, tag="ps")
                nc.tensor.matmul(ps[:, :], xt_tiles[tt][:, 0, off:off + 128],
                                 wpf_sb[:, 0, :], start=True, stop=False)
                nc.tensor.matmul(ps[:, :], xt_tiles[tt][:, 1, off:off + 128],
                                 wpf_sb[:, 1, :], start=False, stop=True)
                st = stp.tile([128, 512], BF16, tag="st")
                st_tiles[b] = st
                nc.scalar.activation(st[:, :], ps[:, :], AF.Copy)

            def emit_S(b):
                S = sp.tile([128, 384], BF16, tag="S")
                S_tiles[b] = S
                nc.vector.tensor_scalar(S[:, :], rel_sb[:, b, :],
                                        iota_sb[:, 0:1], None, OP.is_equal)

            def emit_T(tt):
                pcs = _pieces(tt, nR, T, pad)
                v = vp.tile([128, 2, 516], BF16, tag="v")
                v_tiles[tt] = v
                for ob in range(2):
                    pw = pw_ps.tile([128, 512], F32, tag="ps")
                    nc.tensor.matmul(pw[:, :], wcc_sb[:, 0, ob, :],
                                     xt_tiles[tt][:, 0, :],
                                     start=True, stop=False)
                    for (j, ra, rb, pa) in pcs:
                        half = 0 if ra < 192 else 256
                        nc.tensor.matmul(
                            pw[:, pa:pa + rb - ra],
                            st_tiles[j][:, half + ob * 128:
                                        half + ob * 128 + 128],
                            S_tiles[j][:, ra:rb], start=False, stop=False)
                    nc.tensor.matmul(pw[:, :], wcc_sb[:, 1, ob, :],
                                     xt_tiles[tt][:, 1, :],
                                     start=False, stop=True)
                    if has_b1:
                        nc.scalar.activation(v[:, ob, 1:513], pw[:, :],
                                             AF.Prelu, alpha=SLOPE,
                                             bias=b1_sb[:, ob:ob + 1])
                    else:
                        nc.scalar.activation(v[:, ob, 1:513], pw[:, :],
                                             AF.Prelu, alpha=SLOPE)
                if tt == 0:
                    nc.vector.memset(v[:, :, 0:1], 0.0)
                else:
                    nc.vector.tensor_copy(v[:, :, 0:1],
                                          v_tiles[tt - 1][:, :, 512:513])
                    nc.vector.tensor_copy(v_tiles[tt - 1][:, :, 513:514],
                                          v[:, :, 1:2])
                    emit_conv3(tt - 1, wa_sb, b3_sb, i == NITER - 1)
                    if i + 1 < NITER:
                        xt_next[tt - 1] = emit_A(tt - 1)

            for b in range(nR + LAG):
                if b < nR:
                    if i == 0:
                        # stream in the remaining iteration-0 rel chunks
                        if b in rel0_sched:
                            load_rel_blocks(rel0, 0, *rel0_sched[b])
                        if b < 4:
                            # blockwise lrelu of tile 0 keeps the first
                            # B matmuls fed during the initial x stream
                            if b == 0:
                                xt0 = xtp.tile([128, 2, 512], BF16,
                                               tag="xt")
                                xt_tiles[0] = xt0
                            emit_A0_block(b)
                        else:
                            # lazy lrelu with one-tile lookahead
                            for t2 in (b // 4, b // 4 + 1):
                                if t2 < nT and xt_tiles[t2] is None:
                                    xt_tiles[t2] = emit_A(t2)
                    emit_B(b)
                    # iteration 0 defers the first S-builds until the
                    # tile-0/1 lrelus are emitted, so a late rel chunk 0
                    # cannot head-block the B-stage warmup on the DVE
                    if i != 0 or b > 4:
                        emit_S(b)
                    elif b == 4:
                        for bb in range(5):
                            emit_S(bb)
                    if i + 1 < NITER:
                        # prefetch next iteration's weights + rel, spaced
                        # out so the scalar queue never backs up
                        if b == 8:
                            nxt_w["wpf"] = load_w(i + 1, "wpf")
                        elif b == 10:
                            nxt_w["wcc"] = load_w(i + 1, "wcc")
                        elif b == 12:
                            nxt_w["wa"] = load_w(i + 1, "wa")
                        elif b == 14:
                            nxt_w["b3"] = load_w(i + 1, "b3")
                            if has_b1:
                                nxt_w["b1"] = load_w(i + 1, "b1")
                        elif b in nxt_sched:
                            if nxt_rel is None:
                                nxt_rel = relp.tile([128, nR, 384],
                                                    mybir.dt.int8, tag="rel")
                            load_rel_blocks(nxt_rel, i + 1, *nxt_sched[b])
                if b >= LAG and (b - LAG) % 4 == 0:
                    emit_T((b - LAG) // 4)
            nc.vector.memset(v_tiles[nT - 1][:, :, 513:514], 0.0)
            emit_conv3(nT - 1, wa_sb, b3_sb, i == NITER - 1)
            if i + 1 < NITER:
                xt_next[nT - 1] = emit_A(nT - 1)

    nc.compile()
    return nc


def _to_bf16(a):
    return np.asarray(a, dtype=np.float32).astype(ml_dtypes.bfloat16)


def prep_in_maps(x, d, WC, bC, WP, bP, WF, bF, WA, bA, T=T_FULL):
    """Build the 8 per-core input maps from the full-problem arrays.
    Returns (in_maps, has_b1)."""
    x = np.asarray(x, dtype=np.float32)
    d = np.asarray(d, dtype=np.float32)
    WC, WP, WF, WA = (np.asarray(w, dtype=np.float32) for w in (WC, WP, WF, WA))
    bC, bP, bF, bA = (np.asarray(b, dtype=np.float32) for b in (bC, bP, bF, bA))
    nb = x.shape[0]
    nR = T // 128

    # weights stored in DRAM in the exact SBUF layout (partition dim first)
    wpf = np.empty((NITER, 128, 2, 512), np.float32)
    wcc = np.empty((NITER, 128, 2, 2, 128), np.float32)
    wa = np.empty((NITER, 128, 3, 2, 2, 128), np.float32)
    for i in range(NITER):
        wpfT = np.concatenate([WP[i].T, WF[i].T], axis=1)  # [c', 512]
        wpf[i] = wpfT.reshape(2, 128, 512).transpose(1, 0, 2)
        for cb in range(2):
            for ob in range(2):
                wcc[i, :, cb, ob] = \
                    WC[i][ob * 128:(ob + 1) * 128,
                          cb * 128:(cb + 1) * 128].T
        for k in range(3):
            waT = WA[i, :, :, k].T                         # [c', o]
            wa[i, :, k] = waT.reshape(2, 128, 2, 128) \
                .transpose(1, 0, 2, 3)
    b1 = (bC + bP + bF).astype(np.float32)                  # [NITER, 256]
    has_b1 = bool(np.any(b1 != 0))
    b3 = bA.reshape(NITER, 2, 128).transpose(0, 2, 1).copy()

    wpf, wcc, wa = _to_bf16(wpf), _to_bf16(wcc), _to_bf16(wa)
    iota = np.arange(128, dtype=np.float32).reshape(128, 1)

    tf = np.arange(T, dtype=np.float32)
    in_maps = []
    for b in range(nb):
        dv = d[b, 0].astype(np.float32)
        rel = np.full((NITER, nR, 384), -128, np.int8)
        for i, dil in enumerate(DILATIONS):
            dd = dv * np.float32(dil)
            idxP = np.round(tf - dd).astype(np.int64)
            idxF = np.round(tf + dd).astype(np.int64)
            for j in range(nR):
                # P window: t in [128j, 128j+192)
                a, e = 128 * j, min(128 * j + 192, T)
                hit = idxP[a:e] // 128 == j
                rel[i, j, 0:e - a] = np.where(
                    hit, idxP[a:e] - 128 * j, -128).astype(np.int8)
                # F window: t in [128j-64, 128j+128)
                w0 = 128 * j - 64
                a, e = max(0, w0), min(128 * j + 128, T)
                hit = idxF[a:e] // 128 == j
                rel[i, j, 192 + a - w0:192 + e - w0] = np.where(
                    hit, idxF[a:e] - 128 * j, -128).astype(np.int8)
        m = {
            "x": _to_bf16(x[b].reshape(2, 128, T)),
            "wpf": wpf, "wcc": wcc, "wa": wa, "b3": b3,
            "rel": np.broadcast_to(rel[:, None], (NITER, 128, nR, 384)).copy(),
            "iota": iota,
        }
        if has_b1:
            m["b1"] = b1.reshape(NITER, 2, 128).transpose(0, 2, 1).copy()
        in_maps.append(m)
    return in_maps, has_b1


_nc_cache = {}


def kernel(**inputs) -> np.ndarray:
    T = inputs["x"].shape[2]
    in_maps, has_b1 = prep_in_maps(**inputs, T=T)
    key = (T, has_b1)
    if key not in _nc_cache:
        _nc_cache[key] = build_nc(T, has_b1=has_b1)
    nc = _nc_cache[key]
    res = run_bass_kernel_spmd(nc, in_maps, core_ids=list(range(8)))
    out = np.stack([np.asarray(res.results[i]["out"], dtype=np.float32)
                    .reshape(C, T) for i in range(8)])
    return out


# revision 27
# speedup vs baseline: 1.0037x; 1.0037x over previous
"""Trainium2 Bass kernel for nn_AdaptiveResBlock (8-core data-parallel).

Reference computation (per batch element b, C=256 channels, T=8192 time):
  for i, dil in enumerate((1, 2, 4)):
      xt = lrelu(x)
      xP, xF = time-gather of xt at round(t -/+ d*dil), zero out-of-range
      xt = WC@xt + WP@xP + WF@xF + biases        (1x1 convs over channels)
      xt = lrelu(xt)
      xt = conv3(xt, WA) + bias
      x = xt + x

Structure used:
  * The time-gather commutes with the 1x1 convs:
    WP @ gather(xt) == gather(WP @ xt).
  * B-stage: u^T = [WP;WF] @ xt computed token-major on the PE
    (xt-stationary matmuls), ACT-copied as bf16 st tiles [128 tok, 512].
  * Channel-major windowed one-hot gather: offsets are bounded by
    16*dilation, so every output token whose P-source lies in token
    block j sits in the (128+pad)-token window [128j, 128j+128+pad)
    with pad = 16*dil (F: [128j-pad, 128j+128)).  One matmul per
    (dir, c-half, src block):
      psum[c, t] += st[j][:, c-slice]^T @ S_dir[j][src, t-window]
    lands the gathered conv result DIRECTLY channel-major in PSUM, on top
    of the WC matmuls accumulating in the same bank -- no PE transposes,
    no separate gather pass.  S is built on DVE via is_equal of
    preloaded window-relative indices (int8, -128 sentinel) against an
    iota column; out-of-range indices match no block => free zero mask.
  * v = Prelu(psum) straight from PSUM (ACT); conv3 runs channel-major
    off v tiles with 1-column halos; residual update and the next
    iteration's lrelu are fused right behind each tile (DVE).
  * Weights are stored in DRAM pre-transposed to the exact SBUF layout
    (partition-major) so every weight DMA is a contiguous per-partition
    row; rel window-index tensors stream in 16-block chunks interleaved
    with compute, and iteration i+1's weights/rel prefetch during
    iteration i.

Sharded data-parallel over B=8 across the 8 NeuronCores; weights
replicated; per-core window-index tensors precomputed from d on host.
"""

import numpy as np
import ml_dtypes
from contextlib import ExitStack

import concourse.bass as bass
import concourse.tile as tile
from concourse import mybir, bacc
from concourse.bass_utils import run_bass_kernel_spmd

F32 = mybir.dt.float32
BF16 = mybir.dt.bfloat16
AF = mybir.ActivationFunctionType
OP = mybir.AluOpType

B, C, T_FULL = 8, 256, 8192
DILATIONS = (1, 2, 4)
PADS = tuple(16 * d for d in DILATIONS)   # max gather offset per iteration
NITER = len(DILATIONS)
SLOPE = 0.1
LAG = 8        # B-stage leads the consume stage by this many token blocks
RCHUNK = 8     # rel DMA chunk, in 128-token blocks


def _pieces(tt, nR, T, pad):
    """Gather matmul pieces for 512-token tile tt at max offset `pad`.
    Returns list of (j, rhs_a, rhs_b, ps_a) with rhs cols into the [128,384]
    S tile (P window at 0, F window at 192) and psum column offset."""
    L, R = 512 * tt, 512 * tt + 512
    out = []
    for j in range(max(0, 4 * tt - 1), min(nR, 4 * tt + 5)):
        # P window: t in [128j, 128j+128+pad)
        a = max(L, 128 * j, 0)
        b = min(R, 128 * j + 128 + pad, T)
        if a < b:
            out.append((j, a - 128 * j, b - 128 * j, a - L))
        # F window: t in [128j-pad, 128j+128); rel cols based at 128j-64
        a = max(L, 128 * j - pad, 0)
        b = min(R, 128 * j + 128, T)
        if a < b:
            w0 = 128 * j - 64
            out.append((j, 192 + a - w0, 192 + b - w0, a - L))
    return out


def build_nc(T=T_FULL, num_devices=8, has_b1=False):
    nT = T // 512            # 512-wide time tiles
    nR = T // 128            # 128-wide token blocks
    rchunk = RCHUNK if nR % RCHUNK == 0 else nR
    nchunks = nR // rchunk

    nc = bacc.Bacc("TRN2", target_bir_lowering=False, debug=False,
                   num_devices=num_devices)
    x_d = nc.declare_dram_parameter("x", [2, 128, T], BF16, isOutput=False)
    wpf_d = nc.declare_dram_parameter("wpf", [NITER, 128, 2, 512], BF16,
                                      isOutput=False)
    wcc_d = nc.declare_dram_parameter("wcc", [NITER, 128, 2, 2, 128], BF16,
                                      isOutput=False)
    wa_d = nc.declare_dram_parameter("wa", [NITER, 128, 3, 2, 2, 128], BF16,
                                     isOutput=False)
    b3_d = nc.declare_dram_parameter("b3", [NITER, 128, 2], F32,
                                     isOutput=False)
    rel_d = nc.declare_dram_parameter("rel", [NITER, 128, nR, 384],
                                      mybir.dt.int8, isOutput=False)
    iota_d = nc.declare_dram_parameter("iota", [128, 1], F32, isOutput=False)
    if has_b1:
        b1_d = nc.declare_dram_parameter("b1", [NITER, 128, 2], F32,
                                         isOutput=False)
    out_d = nc.declare_dram_parameter("out", [2, 128, T], F32, isOutput=True)

    with tile.TileContext(nc) as tc, ExitStack() as ctx:
        xpool = ctx.enter_context(tc.tile_pool(name="xres", bufs=1))
        stg = ctx.enter_context(tc.tile_pool(name="stg", bufs=4))
        stp = ctx.enter_context(tc.tile_pool(name="stp", bufs=13))
        sp = ctx.enter_context(tc.tile_pool(name="sp", bufs=13))
        relp = ctx.enter_context(tc.tile_pool(name="relp", bufs=2))
        xtp = ctx.enter_context(tc.tile_pool(name="xtp", bufs=16))
        vp = ctx.enter_context(tc.tile_pool(name="vp", bufs=4))
        wts = ctx.enter_context(tc.tile_pool(name="wts", bufs=2))
        cst = ctx.enter_context(tc.tile_pool(name="cst", bufs=1))
        ps_ps = ctx.enter_context(tc.tile_pool(name="ps", bufs=8, space="PSUM"))
        pu_ps = pw_ps = pc_ps = ps_ps

        # all parameter DMAs dispatch from the sync engine: the scalar
        # (ACT) engine must stay dispatch-free, or flow-control waits on
        # DMA semaphore slots stall its activation stream
        def load_rel_blocks(rel_sb, i, b0, nblk):
            sl = bass.ds(b0, nblk)
            nc.sync.dma_start(rel_sb[:, sl, :], rel_d[i][:, sl, :])

        def load_w(i, what):
            # contiguous per-partition rows: fast, small DMAs
            if what == "wpf":
                t = wts.tile([128, 2, 512], BF16, tag="wpf")
                nc.sync.dma_start(t[:, :, :], wpf_d[i])
            elif what == "wcc":
                t = wts.tile([128, 2, 2, 128], BF16, tag="wcc")
                nc.sync.dma_start(t[:, :, :, :], wcc_d[i])
            elif what == "wa":
                t = wts.tile([128, 3, 2, 2, 128], BF16, tag="wa")
                nc.sync.dma_start(t[:, :, :, :, :], wa_d[i])
            elif what == "b3":
                t = wts.tile([128, 2], F32, tag="b3")
                nc.sync.dma_start(t[:, :], b3_d[i])
            elif what == "b1":
                t = wts.tile([128, 2], F32, tag="b1")
                nc.sync.dma_start(t[:, :], b1_d[i])
            return t

        # ---- startup: finest-granularity head of x so the first B matmul
        # can issue as soon as one 128-token block + wpf have landed ----
        # x streams in (and the residual accumulates) in bf16: every matmul
        # consumer is bf16 anyway, and it halves input HBM traffic; only
        # the final output stages through fp32 tiles for the DMA out.
        x_sb = xpool.tile([128, 2, T], BF16)
        for cb in range(2):
            nc.sync.dma_start(x_sb[:, cb, 0:128], x_d[cb, :, 0:128])
        for cb in range(2):
            nc.sync.dma_start(x_sb[:, cb, 128:512], x_d[cb, :, 128:512])
        # iteration-0 weights ride the scalar queue, wpf (B-stage) first
        w0 = {"wpf": load_w(0, "wpf")}
        iota_sb = cst.tile([128, 1], F32)
        nc.sync.dma_start(iota_sb[:, :], iota_d[:, :])
        w0["wcc"] = load_w(0, "wcc")
        rel0 = relp.tile([128, nR, 384], mybir.dt.int8, tag="rel")
        load_rel_blocks(rel0, 0, 0, min(4, nR))
        w0["wa"] = load_w(0, "wa")
        w0["b3"] = load_w(0, "b3")
        if has_b1:
            w0["b1"] = load_w(0, "b1")
        # rest of x on the sync queue
        HEAD = min(2048, T)
        for sg in range((HEAD - 512) // 512):
            sl = bass.ds(512 + sg * 512, 512)
            for cb in range(2):
                nc.sync.dma_start(x_sb[:, cb, sl], x_d[cb, :, sl])
        for sg in range((T - HEAD) // 2048):
            sl = bass.ds(HEAD + sg * 2048, 2048)
            for cb in range(2):
                nc.sync.dma_start(x_sb[:, cb, sl], x_d[cb, :, sl])

        st_tiles = [None] * nR
        S_tiles = [None] * nR
        xt_tiles = [None] * nT
        xt_next = [None] * nT
        v_tiles = [None] * nT

        def emit_A(tt):
            tsl = bass.ts(tt, 512)
            xt = xtp.tile([128, 2, 512], BF16, tag="xt")
            nc.vector.scalar_tensor_tensor(
                xt[:, :, :], x_sb[:, :, tsl], SLOPE, x_sb[:, :, tsl],
                OP.mult, OP.max)
            return xt

        def emit_A0_block(blk):
            # iteration-0 tile 0 is built per 128-token block to shorten
            # the DMA->lrelu->first-matmul critical path
            sl = bass.ds(blk * 128, 128)
            nc.vector.scalar_tensor_tensor(
                xt_tiles[0][:, :, sl], x_sb[:, :, sl], SLOPE, x_sb[:, :, sl],
                OP.mult, OP.max)

        def emit_conv3(tt, wa_sb, b3_sb, last):
            tsl = bass.ts(tt, 512)
            for ob in range(2):
                py = pc_ps.tile([128, 512], F32, tag="ps")
                j = 0
                for k in range(3):
                    for cb in range(2):
                        nc.tensor.matmul(py[:, :], wa_sb[:, k, cb, ob, :],
                                         v_tiles[tt][:, cb, k:k + 512],
                                         start=(j == 0), stop=(j == 5))
                        j += 1
                if last:
                    # final iteration: residual lands in fp32 staging
                    # tiles (DMA cannot convert dtypes), in halves on the
                    # very last tile so the final out-DMA starts early
                    nh = 2 if tt == nT - 1 else 1
                    for h in range(nh):
                        w = 512 // nh
                        hsl = bass.ds(512 * tt + w * h, w)
                        sg = stg.tile([128, 512], F32, tag="stg")
                        nc.vector.scalar_tensor_tensor(
                            sg[:, 0:w], py[:, w * h:w * h + w],
                            b3_sb[:, ob:ob + 1], x_sb[:, ob, hsl],
                            OP.add, OP.add)
                        nc.sync.dma_start(out_d[ob, :, hsl], sg[:, 0:w])
                else:
                    nc.vector.scalar_tensor_tensor(
                        x_sb[:, ob, tsl], py[:, :], b3_sb[:, ob:ob + 1],
                        x_sb[:, ob, tsl], OP.add, OP.add)

        cur_w = cur_rel = None
        nxt_w = nxt_rel = None
        for i in range(NITER):
            if i == 0:
                cur_w, cur_rel = w0, rel0
            else:
                cur_w, cur_rel = nxt_w, nxt_rel
            nxt_w, nxt_rel = {}, None
            wpf_sb, wcc_sb, wa_sb = cur_w["wpf"], cur_w["wcc"], cur_w["wa"]
            b3_sb, rel_sb = cur_w["b3"], cur_rel
            b1_sb = cur_w.get("b1")
            pad = PADS[i]

            if i != 0:
                xt_tiles, xt_next = xt_next, [None] * nT

            # rel DMA schedule: keep rel traffic out of the early x-stream
            # bandwidth window; each chunk still lands well before its
            # S-builds are consumed LAG blocks later
            rel0_sched = {}
            nxt_sched = {}
            if nR > 4:
                rel0_sched[1] = (4, min(4, nR - 4))
            for k in range(1, (nR + rchunk - 1) // rchunk):
                rel0_sched[5 * k + 3] = (rchunk * k,
                                         min(rchunk, nR - rchunk * k))
            for k in range(0, (nR + rchunk - 1) // rchunk):
                nxt_sched[40 + 3 * k] = (rchunk * k,
                                         min(rchunk, nR - rchunk * k))

            def emit_B(b):
                tt, off = b // 4, (b % 4) * 128
                ps = pu_ps.tile([128, 512], F32, tag="ps")
                nc.tensor.matmul(ps[:, :], xt_tiles[tt][:, 0, off:off + 128],
                                 wpf_sb[:, 0, :], start=True, stop=False)
                nc.tensor.matmul(ps[:, :], xt_tiles[tt][:, 1, off:off + 128],
                                 wpf_sb[:, 1, :], start=False, stop=True)
                st = stp.tile([128, 512], BF16, tag="st")
                st_tiles[b] = st
                nc.scalar.activation(st[:, :], ps[:, :], AF.Copy)

            def emit_S(b):
                S = sp.tile([128, 384], BF16, tag="S")
                S_tiles[b] = S
                nc.vector.tensor_scalar(S[:, :], rel_sb[:, b, :],
                                        iota_sb[:, 0:1], None, OP.is_equal)

            def emit_T(tt):
                pcs = _pieces(tt, nR, T, pad)
                v = vp.tile([128, 2, 516], BF16, tag="v")
                v_tiles[tt] = v
                for ob in range(2):
                    pw = pw_ps.tile([128, 512], F32, tag="ps")
                    nc.tensor.matmul(pw[:, :], wcc_sb[:, 0, ob, :],
                                     xt_tiles[tt][:, 0, :],
                                     start=True, stop=False)
                    for (j, ra, rb, pa) in pcs:
                        half = 0 if ra < 192 else 256
                        nc.tensor.matmul(
                            pw[:, pa:pa + rb - ra],
                            st_tiles[j][:, half + ob * 128:
                                        half + ob * 128 + 128],
                            S_tiles[j][:, ra:rb], start=False, stop=False)
                    nc.tensor.matmul(pw[:, :], wcc_sb[:, 1, ob, :],
                                     xt_tiles[tt][:, 1, :],
                                     start=False, stop=True)
                    if has_b1:
                        nc.scalar.activation(v[:, ob, 1:513], pw[:, :],
                                             AF.Prelu, alpha=SLOPE,
                                             bias=b1_sb[:, ob:ob + 1])
                    else:
                        nc.scalar.activation(v[:, ob, 1:513], pw[:, :],
                                             AF.Prelu, alpha=SLOPE)
                if tt == 0:
                    nc.vector.memset(v[:, :, 0:1], 0.0)
                else:
                    nc.vector.tensor_copy(v[:, :, 0:1],
                                          v_tiles[tt - 1][:, :, 512:513])
                    nc.vector.tensor_copy(v_tiles[tt - 1][:, :, 513:514],
                                          v[:, :, 1:2])
                    emit_conv3(tt - 1, wa_sb, b3_sb, i == NITER - 1)
                    if i + 1 < NITER:
                        xt_next[tt - 1] = emit_A(tt - 1)

            for b in range(nR + LAG):
                if b < nR:
                    if i == 0:
                        # stream in the remaining iteration-0 rel chunks
                        if b in rel0_sched:
                            load_rel_blocks(rel0, 0, *rel0_sched[b])
                        if b < 4:
                            # blockwise lrelu of tile 0 keeps the first
                            # B matmuls fed during the initial x stream
                            if b == 0:
                                xt0 = xtp.tile([128, 2, 512], BF16,
                                               tag="xt")
                                xt_tiles[0] = xt0
                            emit_A0_block(b)
                        else:
                            # lazy lrelu with one-tile lookahead
                            for t2 in (b // 4, b // 4 + 1):
                                if t2 < nT and xt_tiles[t2] is None:
                                    xt_tiles[t2] = emit_A(t2)
                    emit_B(b)
                    # iteration 0 defers the first S-builds until the
                    # tile-0/1 lrelus are emitted, so a late rel chunk 0
                    # cannot head-block the B-stage warmup on the DVE
                    if i != 0 or b > 4:
                        emit_S(b)
                    elif b == 4:
                        for bb in range(5):
                            emit_S(bb)
                    if i + 1 < NITER:
                        # prefetch next iteration's weights + rel, spaced
                        # out so the scalar queue never backs up
                        if b == 8:
                            nxt_w["wpf"] = load_w(i + 1, "wpf")
                        elif b == 10:
                            nxt_w["wcc"] = load_w(i + 1, "wcc")
                        elif b == 12:
                            nxt_w["wa"] = load_w(i + 1, "wa")
                        elif b == 14:
                            nxt_w["b3"] = load_w(i + 1, "b3")
                            if has_b1:
                                nxt_w["b1"] = load_w(i + 1, "b1")
                        elif b in nxt_sched:
                            if nxt_rel is None:
                                nxt_rel = relp.tile([128, nR, 384],
                                                    mybir.dt.int8, tag="rel")
                            load_rel_blocks(nxt_rel, i + 1, *nxt_sched[b])
                if b >= LAG and (b - LAG) % 4 == 0:
                    emit_T((b - LAG) // 4)
            nc.vector.memset(v_tiles[nT - 1][:, :, 513:514], 0.0)
            emit_conv3(nT - 1, wa_sb, b3_sb, i == NITER - 1)
            if i + 1 < NITER:
                xt_next[nT - 1] = emit_A(nT - 1)

    nc.compile()
    return nc


def _to_bf16(a):
    return np.asarray(a, dtype=np.float32).astype(ml_dtypes.bfloat16)


def prep_in_maps(x, d, WC, bC, WP, bP, WF, bF, WA, bA, T=T_FULL):
    """Build the 8 per-core input maps from the full-problem arrays.
    Returns (in_maps, has_b1)."""
    x = np.asarray(x, dtype=np.float32)
    d = np.asarray(d, dtype=np.float32)
    WC, WP, WF, WA = (np.asarray(w, dtype=np.float32) for w in (WC, WP, WF, WA))
    bC, bP, bF, bA = (np.asarray(b, dtype=np.float32) for b in (bC, bP, bF, bA))
    nb = x.shape[0]
    nR = T // 128

    # weights stored in DRAM in the exact SBUF layout (partition dim first)
    wpf = np.empty((NITER, 128, 2, 512), np.float32)
    wcc = np.empty((NITER, 128, 2, 2, 128), np.float32)
    wa = np.empty((NITER, 128, 3, 2, 2, 128), np.float32)
    for i in range(NITER):
        wpfT = np.concatenate([WP[i].T, WF[i].T], axis=1)  # [c', 512]
        wpf[i] = wpfT.reshape(2, 128, 512).transpose(1, 0, 2)
        for cb in range(2):
            for ob in range(2):
                wcc[i, :, cb, ob] = \
                    WC[i][ob * 128:(ob + 1) * 128,
                          cb * 128:(cb + 1) * 128].T
        for k in range(3):
            waT = WA[i, :, :, k].T                         # [c', o]
            wa[i, :, k] = waT.reshape(2, 128, 2, 128) \
                .transpose(1, 0, 2, 3)
    b1 = (bC + bP + bF).astype(np.float32)                  # [NITER, 256]
    has_b1 = bool(np.any(b1 != 0))
    b3 = bA.reshape(NITER, 2, 128).transpose(0, 2, 1).copy()

    wpf, wcc, wa = _to_bf16(wpf), _to_bf16(wcc), _to_bf16(wa)
    iota = np.arange(128, dtype=np.float32).reshape(128, 1)

    tf = np.arange(T, dtype=np.float32)
    in_maps = []
    for b in range(nb):
        dv = d[b, 0].astype(np.float32)
        rel = np.full((NITER, nR, 384), -128, np.int8)
        for i, dil in enumerate(DILATIONS):
            dd = dv * np.float32(dil)
            idxP = np.round(tf - dd).astype(np.int64)
            idxF = np.round(tf + dd).astype(np.int64)
            for j in range(nR):
                # P window: t in [128j, 128j+192)
                a, e = 128 * j, min(128 * j + 192, T)
                hit = idxP[a:e] // 128 == j
                rel[i, j, 0:e - a] = np.where(
                    hit, idxP[a:e] - 128 * j, -128).astype(np.int8)
                # F window: t in [128j-64, 128j+128)
                w0 = 128 * j - 64
                a, e = max(0, w0), min(128 * j + 128, T)
                hit = idxF[a:e] // 128 == j
                rel[i, j, 192 + a - w0:192 + e - w0] = np.where(
                    hit, idxF[a:e] - 128 * j, -128).astype(np.int8)
        m = {
            "x": _to_bf16(x[b].reshape(2, 128, T)),
            "wpf": wpf, "wcc": wcc, "wa": wa, "b3": b3,
            "rel": np.broadcast_to(rel[:, None], (NITER, 128, nR, 384)).copy(),
            "iota": iota,
        }
        if has_b1:
            m["b1"] = b1.reshape(NITER, 2, 128).transpose(0, 2, 1).copy()
        in_maps.append(m)
    return in_maps, has_b1


_nc_cache = {}


def kernel(**inputs) -> np.ndarray:
    T = inputs["x"].shape[2]
    in_maps, has_b1 = prep_in_maps(**inputs, T=T)
    key = (T, has_b1)
    if key not in _nc_cache:
        _nc_cache[key] = build_nc(T, has_b1=has_b1)
    nc = _nc_cache[key]
    res = run_bass_kernel_spmd(nc, in_maps, core_ids=list(range(8)))
    out = np.stack([np.asarray(res.results[i]["out"], dtype=np.float32)
                    .reshape(C, T) for i in range(8)])
    return out


# revision 33
# speedup vs baseline: 1.0200x; 1.0162x over previous
"""Trainium2 Bass kernel for nn_AdaptiveResBlock (8-core data-parallel).

Reference computation (per batch element b, C=256 channels, T=8192 time):
  for i, dil in enumerate((1, 2, 4)):
      xt = lrelu(x)
      xP, xF = time-gather of xt at round(t -/+ d*dil), zero out-of-range
      xt = WC@xt + WP@xP + WF@xF + biases        (1x1 convs over channels)
      xt = lrelu(xt)
      xt = conv3(xt, WA) + bias
      x = xt + x

Structure used:
  * The time-gather commutes with the 1x1 convs:
    WP @ gather(xt) == gather(WP @ xt).
  * B-stage: u^T = [WP;WF] @ xt computed token-major on the PE
    (xt-stationary matmuls), ACT-copied as bf16 st tiles [128 tok, 512].
  * Channel-major windowed one-hot gather: offsets are bounded by
    16*dilation, so every output token whose P-source lies in token
    block j sits in the (128+pad)-token window [128j, 128j+128+pad)
    with pad = 16*dil (F: [128j-pad, 128j+128)).  One matmul per
    (dir, c-half, src block):
      psum[c, t] += st[j][:, c-slice]^T @ S_dir[j][src, t-window]
    lands the gathered conv result DIRECTLY channel-major in PSUM, on top
    of the WC matmuls accumulating in the same bank -- no PE transposes,
    no separate gather pass.  S is built on DVE via is_equal of
    preloaded window-relative indices (int8, -128 sentinel) against an
    iota column; out-of-range indices match no block => free zero mask.
  * v = Prelu(psum) straight from PSUM (ACT); conv3 runs channel-major
    off v tiles with 1-column halos; residual update and the next
    iteration's lrelu are fused right behind each tile (DVE).
  * Weights are stored in DRAM pre-transposed to the exact SBUF layout
    (partition-major) so every weight DMA is a contiguous per-partition
    row; rel window-index tensors stream in 16-block chunks interleaved
    with compute, and iteration i+1's weights/rel prefetch during
    iteration i.

Sharded data-parallel over B=8 across the 8 NeuronCores; weights
replicated; per-core window-index tensors precomputed from d on host.
"""

import numpy as np
import ml_dtypes
from contextlib import ExitStack

import concourse.bass as bass
import concourse.tile as tile
from concourse import mybir, bacc
from concourse.bass_utils import run_bass_kernel_spmd

F32 = mybir.dt.float32
BF16 = mybir.dt.bfloat16
AF = mybir.ActivationFunctionType
OP = mybir.AluOpType

B, C, T_FULL = 8, 256, 8192
DILATIONS = (1, 2, 4)
PADS = tuple(16 * d for d in DILATIONS)   # max gather offset per iteration
NITER = len(DILATIONS)
SLOPE = 0.1
LAG = 8        # B-stage leads the consume stage by this many token blocks
RCHUNK = 8     # rel DMA chunk, in 128-token blocks


def _pieces(tt, nR, T, pad):
    """Gather matmul pieces for 512-token tile tt at max offset `pad`.
    Returns list of (j, rhs_a, rhs_b, ps_a) with rhs cols into the [128,384]
    S tile (P window at 0, F window at 192) and psum column offset."""
    L, R = 512 * tt, 512 * tt + 512
    out = []
    for j in range(max(0, 4 * tt - 1), min(nR, 4 * tt + 5)):
        # P window: t in [128j, 128j+128+pad)
        a = max(L, 128 * j, 0)
        b = min(R, 128 * j + 128 + pad, T)
        if a < b:
            out.append((j, a - 128 * j, b - 128 * j, a - L))
        # F window: t in [128j-pad, 128j+128); rel cols based at 128j-64
        a = max(L, 128 * j - pad, 0)
        b = min(R, 128 * j + 128, T)
        if a < b:
            w0 = 128 * j - 64
            out.append((j, 192 + a - w0, 192 + b - w0, a - L))
    return out


def build_nc(T=T_FULL, num_devices=8, has_b1=False):
    nT = T // 512            # 512-wide time tiles
    nR = T // 128            # 128-wide token blocks
    rchunk = RCHUNK if nR % RCHUNK == 0 else nR
    nchunks = nR // rchunk

    nc = bacc.Bacc("TRN2", target_bir_lowering=False, debug=False,
                   num_devices=num_devices)
    x_d = nc.declare_dram_parameter("x", [2, 128, T], BF16, isOutput=False)
    wpf_d = nc.declare_dram_parameter("wpf", [NITER, 128, 2, 512], BF16,
                                      isOutput=False)
    wcc_d = nc.declare_dram_parameter("wcc", [NITER, 128, 2, 2, 128], BF16,
                                      isOutput=False)
    wa_d = nc.declare_dram_parameter("wa", [NITER, 128, 3, 2, 2, 128], BF16,
                                     isOutput=False)
    b3_d = nc.declare_dram_parameter("b3", [NITER, 128, 2], F32,
                                     isOutput=False)
    rel_d = nc.declare_dram_parameter("rel", [NITER, 128, nR, 384],
                                      mybir.dt.int8, isOutput=False)
    iota_d = nc.declare_dram_parameter("iota", [128, 1], F32, isOutput=False)
    if has_b1:
        b1_d = nc.declare_dram_parameter("b1", [NITER, 128, 2], F32,
                                         isOutput=False)
    out_d = nc.declare_dram_parameter("out", [2, 128, T], F32, isOutput=True)

    with tile.TileContext(nc) as tc, ExitStack() as ctx:
        xpool = ctx.enter_context(tc.tile_pool(name="xres", bufs=1))
        stg = ctx.enter_context(tc.tile_pool(name="stg", bufs=4))
        stp = ctx.enter_context(tc.tile_pool(name="stp", bufs=13))
        sp = ctx.enter_context(tc.tile_pool(name="sp", bufs=13))
        relp = ctx.enter_context(tc.tile_pool(name="relp", bufs=2))
        xtp = ctx.enter_context(tc.tile_pool(name="xtp", bufs=16))
        vp = ctx.enter_context(tc.tile_pool(name="vp", bufs=4))
        wts = ctx.enter_context(tc.tile_pool(name="wts", bufs=2))
        cst = ctx.enter_context(tc.tile_pool(name="cst", bufs=1))
        ps_ps = ctx.enter_context(tc.tile_pool(name="ps", bufs=8, space="PSUM"))
        pu_ps = pw_ps = pc_ps = ps_ps

        # all parameter DMAs dispatch from the sync engine: the scalar
        # (ACT) engine must stay dispatch-free, or flow-control waits on
        # DMA semaphore slots stall its activation stream
        def load_rel_blocks(rel_sb, i, b0, nblk):
            sl = bass.ds(b0, nblk)
            nc.sync.dma_start(rel_sb[:, sl, :], rel_d[i][:, sl, :])

        def load_w(i, what):
            # contiguous per-partition rows: fast, small DMAs
            if what == "wpf":
                t = wts.tile([128, 2, 512], BF16, tag="wpf")
                nc.sync.dma_start(t[:, :, :], wpf_d[i])
            elif what == "wcc":
                t = wts.tile([128, 2, 2, 128], BF16, tag="wcc")
                nc.sync.dma_start(t[:, :, :, :], wcc_d[i])
            elif what == "wa":
                t = wts.tile([128, 3, 2, 2, 128], BF16, tag="wa")
                nc.sync.dma_start(t[:, :, :, :, :], wa_d[i])
            elif what == "b3":
                t = wts.tile([128, 2], F32, tag="b3")
                nc.sync.dma_start(t[:, :], b3_d[i])
            elif what == "b1":
                t = wts.tile([128, 2], F32, tag="b1")
                nc.sync.dma_start(t[:, :], b1_d[i])
            return t

        # ---- startup ----
        # Everything dispatches from the sync engine, interleaved in
        # NEED-time order: the DMA semaphore-slot flow control then paces
        # later transfers behind earlier ones, which is exactly the
        # priority we want.  x streams in (and the residual accumulates)
        # in bf16: every matmul consumer is bf16 anyway, and it halves
        # input HBM traffic; only the final output stages through fp32
        # tiles for the DMA out.
        x_sb = xpool.tile([128, 2, T], BF16)

        def load_x(a, b):
            sl = bass.ds(a, b - a)
            for cb in range(2):
                nc.sync.dma_start(x_sb[:, cb, sl], x_d[cb, :, sl])

        def load_x_clamped(a, b):
            if a < T:
                load_x(a, min(b, T))

        load_x(0, 128)
        w0 = {"wpf": load_w(0, "wpf")}
        load_x(128, 512)
        iota_sb = cst.tile([128, 1], F32)
        nc.sync.dma_start(iota_sb[:, :], iota_d[:, :])
        rel0 = relp.tile([128, nR, 384], mybir.dt.int8, tag="rel")
        load_rel_blocks(rel0, 0, 0, min(8, nR))
        w0["wcc"] = load_w(0, "wcc")
        load_x_clamped(512, 1536)
        if nR > 8:
            load_rel_blocks(rel0, 0, 8, min(8, nR - 8))
        w0["wa"] = load_w(0, "wa")
        load_x_clamped(1536, 3584)
        if nR > 16:
            load_rel_blocks(rel0, 0, 16, min(16, nR - 16))
        w0["b3"] = load_w(0, "b3")
        if has_b1:
            w0["b1"] = load_w(0, "b1")
        load_x_clamped(3584, 5632)
        if nR > 32:
            load_rel_blocks(rel0, 0, 32, min(16, nR - 32))
        load_x_clamped(5632, T)
        if nR > 48:
            load_rel_blocks(rel0, 0, 48, nR - 48)

        st_tiles = [None] * nR
        S_tiles = [None] * nR
        xt_tiles = [None] * nT
        xt_next = [None] * nT
        v_tiles = [None] * nT

        def emit_A(tt):
            tsl = bass.ts(tt, 512)
            xt = xtp.tile([128, 2, 512], BF16, tag="xt")
            nc.vector.scalar_tensor_tensor(
                xt[:, :, :], x_sb[:, :, tsl], SLOPE, x_sb[:, :, tsl],
                OP.mult, OP.max)
            return xt

        def emit_A0_block(blk):
            # iteration-0 tile 0 is built per 128-token block to shorten
            # the DMA->lrelu->first-matmul critical path
            sl = bass.ds(blk * 128, 128)
            nc.vector.scalar_tensor_tensor(
                xt_tiles[0][:, :, sl], x_sb[:, :, sl], SLOPE, x_sb[:, :, sl],
                OP.mult, OP.max)

        def emit_conv3(tt, wa_sb, b3_sb, last):
            tsl = bass.ts(tt, 512)
            for ob in range(2):
                py = pc_ps.tile([128, 512], F32, tag="ps")
                j = 0
                for k in range(3):
                    for cb in range(2):
                        nc.tensor.matmul(py[:, :], wa_sb[:, k, cb, ob, :],
                                         v_tiles[tt][:, cb, k:k + 512],
                                         start=(j == 0), stop=(j == 5))
                        j += 1
                if last:
                    # final iteration: residual lands in fp32 staging
                    # tiles (DMA cannot convert dtypes), in halves on the
                    # very last tile so the final out-DMA starts early
                    nh = 2 if tt == nT - 1 else 1
                    for h in range(nh):
                        w = 512 // nh
                        hsl = bass.ds(512 * tt + w * h, w)
                        sg = stg.tile([128, 512], F32, tag="stg")
                        nc.vector.scalar_tensor_tensor(
                            sg[:, 0:w], py[:, w * h:w * h + w],
                            b3_sb[:, ob:ob + 1], x_sb[:, ob, hsl],
                            OP.add, OP.add)
                        nc.sync.dma_start(out_d[ob, :, hsl], sg[:, 0:w])
                else:
                    nc.vector.scalar_tensor_tensor(
                        x_sb[:, ob, tsl], py[:, :], b3_sb[:, ob:ob + 1],
                        x_sb[:, ob, tsl], OP.add, OP.add)

        cur_w = cur_rel = None
        nxt_w = nxt_rel = None
        for i in range(NITER):
            if i == 0:
                cur_w, cur_rel = w0, rel0
            else:
                cur_w, cur_rel = nxt_w, nxt_rel
            nxt_w, nxt_rel = {}, None
            wpf_sb, wcc_sb, wa_sb = cur_w["wpf"], cur_w["wcc"], cur_w["wa"]
            b3_sb, rel_sb = cur_w["b3"], cur_rel
            b1_sb = cur_w.get("b1")
            pad = PADS[i]

            if i != 0:
                xt_tiles, xt_next = xt_next, [None] * nT

            def emit_B(b):
                tt, off = b // 4, (b % 4) * 128
                ps = pu_ps.tile([128, 512], F32, tag="ps")
                nc.tensor.matmul(ps[:, :], xt_tiles[tt][:, 0, off:off + 128],
                                 wpf_sb[:, 0, :], start=True, stop=False)
                nc.tensor.matmul(ps[:, :], xt_tiles[tt][:, 1, off:off + 128],
                                 wpf_sb[:, 1, :], start=False, stop=True)
                st = stp.tile([128, 512], BF16, tag="st")
                st_tiles[b] = st
                nc.scalar.activation(st[:, :], ps[:, :], AF.Copy)

            def emit_S(b):
                S = sp.tile([128, 384], BF16, tag="S")
                S_tiles[b] = S
                nc.vector.tensor_scalar(S[:, :], rel_sb[:, b, :],
                                        iota_sb[:, 0:1], None, OP.is_equal)

            def emit_T(tt):
                pcs = _pieces(tt, nR, T, pad)
                v = vp.tile([128, 2, 516], BF16, tag="v")
                v_tiles[tt] = v
                for ob in range(2):
                    pw = pw_ps.tile([128, 512], F32, tag="ps")
                    nc.tensor.matmul(pw[:, :], wcc_sb[:, 0, ob, :],
                                     xt_tiles[tt][:, 0, :],
                                     start=True, stop=False)
                    for (j, ra, rb, pa) in pcs:
                        half = 0 if ra < 192 else 256
                        nc.tensor.matmul(
                            pw[:, pa:pa + rb - ra],
                            st_tiles[j][:, half + ob * 128:
                                        half + ob * 128 + 128],
                            S_tiles[j][:, ra:rb], start=False, stop=False)
                    nc.tensor.matmul(pw[:, :], wcc_sb[:, 1, ob, :],
                                     xt_tiles[tt][:, 1, :],
                                     start=False, stop=True)
                    if has_b1:
                        nc.scalar.activation(v[:, ob, 1:513], pw[:, :],
                                             AF.Prelu, alpha=SLOPE,
                                             bias=b1_sb[:, ob:ob + 1])
                    else:
                        nc.scalar.activation(v[:, ob, 1:513], pw[:, :],
                                             AF.Prelu, alpha=SLOPE)
                if tt == 0:
                    nc.vector.memset(v[:, :, 0:1], 0.0)
                else:
                    nc.vector.tensor_copy(v[:, :, 0:1],
                                          v_tiles[tt - 1][:, :, 512:513])
                    nc.vector.tensor_copy(v_tiles[tt - 1][:, :, 513:514],
                                          v[:, :, 1:2])
                    emit_conv3(tt - 1, wa_sb, b3_sb, i == NITER - 1)
                    if i + 1 < NITER:
                        xt_next[tt - 1] = emit_A(tt - 1)

            for b in range(nR + LAG):
                if b < nR:
                    if i == 0:
                        if b < 4:
                            # blockwise lrelu of tile 0 keeps the first
                            # B matmuls fed during the initial x stream
                            if b == 0:
                                xt0 = xtp.tile([128, 2, 512], BF16,
                                               tag="xt")
                                xt_tiles[0] = xt0
                            emit_A0_block(b)
                        else:
                            # lazy lrelu with one-tile lookahead
                            for t2 in (b // 4, b // 4 + 1):
                                if t2 < nT and xt_tiles[t2] is None:
                                    xt_tiles[t2] = emit_A(t2)
                    emit_B(b)
                    # iteration 0 defers the first S-builds until the
                    # tile-0/1 lrelus are emitted, so a late rel chunk 0
                    # cannot head-block the B-stage warmup on the DVE
                    if i != 0 or b > 4:
                        emit_S(b)
                    elif b == 4:
                        for bb in range(5):
                            emit_S(bb)
                    if i + 1 < NITER:
                        # prefetch next iteration's weights + rel on the
                        # (by now idle) sync queue, once the x stream and
                        # iteration-0 rel chunks have fully dispatched
                        if b == 36:
                            nxt_w["wpf"] = load_w(i + 1, "wpf")
                        elif b == 37:
                            nxt_w["wcc"] = load_w(i + 1, "wcc")
                        elif b == 38:
                            nxt_w["wa"] = load_w(i + 1, "wa")
                        elif b == 39:
                            nxt_w["b3"] = load_w(i + 1, "b3")
                            if has_b1:
                                nxt_w["b1"] = load_w(i + 1, "b1")
                        elif b == 40:
                            nxt_rel = relp.tile([128, nR, 384],
                                                mybir.dt.int8, tag="rel")
                            load_rel_blocks(nxt_rel, i + 1, 0, nR)
                if b >= LAG and (b - LAG) % 4 == 0:
                    emit_T((b - LAG) // 4)
            nc.vector.memset(v_tiles[nT - 1][:, :, 513:514], 0.0)
            emit_conv3(nT - 1, wa_sb, b3_sb, i == NITER - 1)
            if i + 1 < NITER:
                xt_next[nT - 1] = emit_A(nT - 1)

    nc.compile()
    return nc


def _to_bf16(a):
    return np.asarray(a, dtype=np.float32).astype(ml_dtypes.bfloat16)


def prep_in_maps(x, d, WC, bC, WP, bP, WF, bF, WA, bA, T=T_FULL):
    """Build the 8 per-core input maps from the full-problem arrays.
    Returns (in_maps, has_b1)."""
    x = np.asarray(x, dtype=np.float32)
    d = np.asarray(d, dtype=np.float32)
    WC, WP, WF, WA = (np.asarray(w, dtype=np.float32) for w in (WC, WP, WF, WA))
    bC, bP, bF, bA = (np.asarray(b, dtype=np.float32) for b in (bC, bP, bF, bA))
    nb = x.shape[0]
    nR = T // 128

    # weights stored in DRAM in the exact SBUF layout (partition dim first)
    wpf = np.empty((NITER, 128, 2, 512), np.float32)
    wcc = np.empty((NITER, 128, 2, 2, 128), np.float32)
    wa = np.empty((NITER, 128, 3, 2, 2, 128), np.float32)
    for i in range(NITER):
        wpfT = np.concatenate([WP[i].T, WF[i].T], axis=1)  # [c', 512]
        wpf[i] = wpfT.reshape(2, 128, 512).transpose(1, 0, 2)
        for cb in range(2):
            for ob in range(2):
                wcc[i, :, cb, ob] = \
                    WC[i][ob * 128:(ob + 1) * 128,
                          cb * 128:(cb + 1) * 128].T
        for k in range(3):
            waT = WA[i, :, :, k].T                         # [c', o]
            wa[i, :, k] = waT.reshape(2, 128, 2, 128) \
                .transpose(1, 0, 2, 3)
    b1 = (bC + bP + bF).astype(np.float32)                  # [NITER, 256]
    has_b1 = bool(np.any(b1 != 0))
    b3 = bA.reshape(NITER, 2, 128).transpose(0, 2, 1).copy()

    wpf, wcc, wa = _to_bf16(wpf), _to_bf16(wcc), _to_bf16(wa)
    iota = np.arange(128, dtype=np.float32).reshape(128, 1)

    tf = np.arange(T, dtype=np.float32)
    in_maps = []
    for b in range(nb):
        dv = d[b, 0].astype(np.float32)
        rel = np.full((NITER, nR, 384), -128, np.int8)
        for i, dil in enumerate(DILATIONS):
            dd = dv * np.float32(dil)
            idxP = np.round(tf - dd).astype(np.int64)
            idxF = np.round(tf + dd).astype(np.int64)
            for j in range(nR):
                # P window: t in [128j, 128j+192)
                a, e = 128 * j, min(128 * j + 192, T)
                hit = idxP[a:e] // 128 == j
                rel[i, j, 0:e - a] = np.where(
                    hit, idxP[a:e] - 128 * j, -128).astype(np.int8)
                # F window: t in [128j-64, 128j+128)
                w0 = 128 * j - 64
                a, e = max(0, w0), min(128 * j + 128, T)
                hit = idxF[a:e] // 128 == j
                rel[i, j, 192 + a - w0:192 + e - w0] = np.where(
                    hit, idxF[a:e] - 128 * j, -128).astype(np.int8)
        m = {
            "x": _to_bf16(x[b].reshape(2, 128, T)),
            "wpf": wpf, "wcc": wcc, "wa": wa, "b3": b3,
            "rel": np.broadcast_to(rel[:, None], (NITER, 128, nR, 384)).copy(),
            "iota": iota,
        }
        if has_b1:
            m["b1"] = b1.reshape(NITER, 2, 128).transpose(0, 2, 1).copy()
        in_maps.append(m)
    return in_maps, has_b1


_nc_cache = {}


def kernel(**inputs) -> np.ndarray:
    T = inputs["x"].shape[2]
    in_maps, has_b1 = prep_in_maps(**inputs, T=T)
    key = (T, has_b1)
    if key not in _nc_cache:
        _nc_cache[key] = build_nc(T, has_b1=has_b1)
    nc = _nc_cache[key]
    res = run_bass_kernel_spmd(nc, in_maps, core_ids=list(range(8)))
    out = np.stack([np.asarray(res.results[i]["out"], dtype=np.float32)
                    .reshape(C, T) for i in range(8)])
    return out


# revision 37
# speedup vs baseline: 1.0244x; 1.0043x over previous
"""Trainium2 Bass kernel for nn_AdaptiveResBlock (8-core data-parallel).

Reference computation (per batch element b, C=256 channels, T=8192 time):
  for i, dil in enumerate((1, 2, 4)):
      xt = lrelu(x)
      xP, xF = time-gather of xt at round(t -/+ d*dil), zero out-of-range
      xt = WC@xt + WP@xP + WF@xF + biases        (1x1 convs over channels)
      xt = lrelu(xt)
      xt = conv3(xt, WA) + bias
      x = xt + x

Structure used:
  * The time-gather commutes with the 1x1 convs:
    WP @ gather(xt) == gather(WP @ xt).
  * B-stage: u^T = [WP;WF] @ xt computed token-major on the PE
    (xt-stationary matmuls), ACT-copied as bf16 st tiles [128 tok, 512].
  * Channel-major windowed one-hot gather: offsets are bounded by
    16*dilation, so every output token whose P-source lies in token
    block j sits in the (128+pad)-token window [128j, 128j+128+pad)
    with pad = 16*dil (F: [128j-pad, 128j+128)).  One matmul per
    (dir, c-half, src block):
      psum[c, t] += st[j][:, c-slice]^T @ S_dir[j][src, t-window]
    lands the gathered conv result DIRECTLY channel-major in PSUM, on top
    of the WC matmuls accumulating in the same bank -- no PE transposes,
    no separate gather pass.  S is built on DVE via is_equal of
    preloaded window-relative indices (int8, -128 sentinel) against an
    iota column; out-of-range indices match no block => free zero mask.
  * v = Prelu(psum) straight from PSUM (ACT); conv3 runs channel-major
    off v tiles with 1-column halos; residual update and the next
    iteration's lrelu are fused right behind each tile (DVE).
  * Weights are stored in DRAM pre-transposed to the exact SBUF layout
    (partition-major) so every weight DMA is a contiguous per-partition
    row; rel window-index tensors stream in 16-block chunks interleaved
    with compute, and iteration i+1's weights/rel prefetch during
    iteration i.

Sharded data-parallel over B=8 across the 8 NeuronCores; weights
replicated; per-core window-index tensors precomputed from d on host.
"""

import numpy as np
import ml_dtypes
from contextlib import ExitStack

import concourse.bass as bass
import concourse.tile as tile
from concourse import mybir, bacc
from concourse.bass_utils import run_bass_kernel_spmd

F32 = mybir.dt.float32
BF16 = mybir.dt.bfloat16
AF = mybir.ActivationFunctionType
OP = mybir.AluOpType

B, C, T_FULL = 8, 256, 8192
DILATIONS = (1, 2, 4)
PADS = tuple(16 * d for d in DILATIONS)   # max gather offset per iteration
NITER = len(DILATIONS)
SLOPE = 0.1
LAG = 8        # B-stage leads the consume stage by this many token blocks
RCHUNK = 8     # rel DMA chunk, in 128-token blocks


def _pieces(tt, nR, T, pad):
    """Gather matmul pieces for 512-token tile tt at max offset `pad`.
    Returns list of (j, rhs_a, rhs_b, ps_a) with rhs cols into the [128,384]
    S tile (P window at 0, F window at 192) and psum column offset."""
    L, R = 512 * tt, 512 * tt + 512
    out = []
    for j in range(max(0, 4 * tt - 1), min(nR, 4 * tt + 5)):
        # P window: t in [128j, 128j+128+pad)
        a = max(L, 128 * j, 0)
        b = min(R, 128 * j + 128 + pad, T)
        if a < b:
            out.append((j, a - 128 * j, b - 128 * j, a - L))
        # F window: t in [128j-pad, 128j+128); rel cols based at 128j-64
        a = max(L, 128 * j - pad, 0)
        b = min(R, 128 * j + 128, T)
        if a < b:
            w0 = 128 * j - 64
            out.append((j, 192 + a - w0, 192 + b - w0, a - L))
    return out


def build_nc(T=T_FULL, num_devices=8, has_b1=False):
    nT = T // 512            # 512-wide time tiles
    nR = T // 128            # 128-wide token blocks
    rchunk = RCHUNK if nR % RCHUNK == 0 else nR
    nchunks = nR // rchunk

    nc = bacc.Bacc("TRN2", target_bir_lowering=False, debug=False,
                   num_devices=num_devices)
    x_d = nc.declare_dram_parameter("x", [2, 128, T], BF16, isOutput=False)
    wpf_d = nc.declare_dram_parameter("wpf", [NITER, 128, 2, 512], BF16,
                                      isOutput=False)
    wcc_d = nc.declare_dram_parameter("wcc", [NITER, 128, 2, 2, 128], BF16,
                                      isOutput=False)
    wa_d = nc.declare_dram_parameter("wa", [NITER, 128, 3, 2, 2, 128], BF16,
                                     isOutput=False)
    b3_d = nc.declare_dram_parameter("b3", [NITER, 128, 2], F32,
                                     isOutput=False)
    rel_d = nc.declare_dram_parameter("rel", [NITER, 128, nR, 384],
                                      mybir.dt.int8, isOutput=False)
    iota_d = nc.declare_dram_parameter("iota", [128, 1], F32, isOutput=False)
    if has_b1:
        b1_d = nc.declare_dram_parameter("b1", [NITER, 128, 2], F32,
                                         isOutput=False)
    out_d = nc.declare_dram_parameter("out", [2, 128, T], F32, isOutput=True)

    with tile.TileContext(nc) as tc, ExitStack() as ctx:
        xpool = ctx.enter_context(tc.tile_pool(name="xres", bufs=1))
        stg = ctx.enter_context(tc.tile_pool(name="stg", bufs=4))
        stp = ctx.enter_context(tc.tile_pool(name="stp", bufs=13))
        sp = ctx.enter_context(tc.tile_pool(name="sp", bufs=13))
        relp = ctx.enter_context(tc.tile_pool(name="relp", bufs=2))
        xtp = ctx.enter_context(tc.tile_pool(name="xtp", bufs=16))
        vp = ctx.enter_context(tc.tile_pool(name="vp", bufs=4))
        wts = ctx.enter_context(tc.tile_pool(name="wts", bufs=2))
        cst = ctx.enter_context(tc.tile_pool(name="cst", bufs=1))
        ps_ps = ctx.enter_context(tc.tile_pool(name="ps", bufs=8, space="PSUM"))
        pu_ps = pw_ps = pc_ps = ps_ps

        # all parameter DMAs dispatch from the sync engine: the scalar
        # (ACT) engine must stay dispatch-free, or flow-control waits on
        # DMA semaphore slots stall its activation stream
        def load_rel_blocks(rel_sb, i, b0, nblk):
            sl = bass.ds(b0, nblk)
            nc.sync.dma_start(rel_sb[:, sl, :], rel_d[i][:, sl, :])

        def load_w(i, what):
            # contiguous per-partition rows: fast, small DMAs
            if what == "wpf":
                t = wts.tile([128, 2, 512], BF16, tag="wpf")
                nc.sync.dma_start(t[:, :, :], wpf_d[i])
            elif what == "wcc":
                t = wts.tile([128, 2, 2, 128], BF16, tag="wcc")
                nc.sync.dma_start(t[:, :, :, :], wcc_d[i])
            elif what == "wa":
                t = wts.tile([128, 3, 2, 2, 128], BF16, tag="wa")
                nc.sync.dma_start(t[:, :, :, :, :], wa_d[i])
            elif what == "b3":
                t = wts.tile([128, 2], F32, tag="b3")
                nc.sync.dma_start(t[:, :], b3_d[i])
            elif what == "b1":
                t = wts.tile([128, 2], F32, tag="b1")
                nc.sync.dma_start(t[:, :], b1_d[i])
            return t

        # ---- startup ----
        # Everything dispatches from the sync engine, interleaved in
        # NEED-time order: the DMA semaphore-slot flow control then paces
        # later transfers behind earlier ones, which is exactly the
        # priority we want.  x streams in (and the residual accumulates)
        # in bf16: every matmul consumer is bf16 anyway, and it halves
        # input HBM traffic; only the final output stages through fp32
        # tiles for the DMA out.
        x_sb = xpool.tile([128, 2, T], BF16)

        def load_x(a, b):
            sl = bass.ds(a, b - a)
            for cb in range(2):
                nc.sync.dma_start(x_sb[:, cb, sl], x_d[cb, :, sl])

        def load_x_clamped(a, b):
            if a < T:
                load_x(a, min(b, T))

        load_x(0, 128)
        w0 = {"wpf": load_w(0, "wpf")}
        load_x(128, 512)
        iota_sb = cst.tile([128, 1], F32)
        nc.sync.dma_start(iota_sb[:, :], iota_d[:, :])
        rel0 = relp.tile([128, nR, 384], mybir.dt.int8, tag="rel")
        load_rel_blocks(rel0, 0, 0, min(8, nR))
        w0["wcc"] = load_w(0, "wcc")
        load_x_clamped(512, 1024)
        if nR > 8:
            load_rel_blocks(rel0, 0, 8, min(8, nR - 8))
        w0["wa"] = load_w(0, "wa")
        load_x_clamped(1024, 2048)
        if nR > 16:
            load_rel_blocks(rel0, 0, 16, min(16, nR - 16))
        w0["b3"] = load_w(0, "b3")
        if has_b1:
            w0["b1"] = load_w(0, "b1")
        load_x_clamped(2048, 4096)
        if nR > 32:
            load_rel_blocks(rel0, 0, 32, min(16, nR - 32))
        load_x_clamped(4096, 6144)
        if nR > 48:
            load_rel_blocks(rel0, 0, 48, nR - 48)
        load_x_clamped(6144, T)

        st_tiles = [None] * nR
        S_tiles = [None] * nR
        xt_tiles = [None] * nT
        xt_next = [None] * nT
        v_tiles = [None] * nT

        def emit_A(tt):
            tsl = bass.ts(tt, 512)
            xt = xtp.tile([128, 2, 512], BF16, tag="xt")
            nc.vector.scalar_tensor_tensor(
                xt[:, :, :], x_sb[:, :, tsl], SLOPE, x_sb[:, :, tsl],
                OP.mult, OP.max)
            return xt

        def emit_A0_block(blk):
            # iteration-0 tile 0 is built per 128-token block to shorten
            # the DMA->lrelu->first-matmul critical path
            sl = bass.ds(blk * 128, 128)
            nc.vector.scalar_tensor_tensor(
                xt_tiles[0][:, :, sl], x_sb[:, :, sl], SLOPE, x_sb[:, :, sl],
                OP.mult, OP.max)

        def emit_conv3(tt, wa_sb, b3_sb, last):
            tsl = bass.ts(tt, 512)
            final = last and tt == nT - 1
            for ob in range(2):
                py = pc_ps.tile([128, 512], F32, tag="ps")
                # the very last tile runs its matmul chain in halves so
                # the drain (STT + out-DMA) overlaps the remaining matmuls
                nh = 2 if final else 1
                w = 512 // nh
                for h in range(nh):
                    j = 0
                    for k in range(3):
                        for cb in range(2):
                            nc.tensor.matmul(
                                py[:, w * h:w * h + w],
                                wa_sb[:, k, cb, ob, :],
                                v_tiles[tt][:, cb, k + w * h:k + w * h + w],
                                start=(j == 0), stop=(j == 5))
                            j += 1
                    if last:
                        # final iteration: residual lands in fp32 staging
                        # tiles (DMA cannot convert dtypes)
                        hsl = bass.ds(512 * tt + w * h, w)
                        sg = stg.tile([128, 512], F32, tag="stg")
                        nc.vector.scalar_tensor_tensor(
                            sg[:, 0:w], py[:, w * h:w * h + w],
                            b3_sb[:, ob:ob + 1], x_sb[:, ob, hsl],
                            OP.add, OP.add)
                        nc.sync.dma_start(out_d[ob, :, hsl], sg[:, 0:w])
                if not last:
                    nc.vector.scalar_tensor_tensor(
                        x_sb[:, ob, tsl], py[:, :], b3_sb[:, ob:ob + 1],
                        x_sb[:, ob, tsl], OP.add, OP.add)

        cur_w = cur_rel = None
        nxt_w = nxt_rel = None
        for i in range(NITER):
            if i == 0:
                cur_w, cur_rel = w0, rel0
            else:
                cur_w, cur_rel = nxt_w, nxt_rel
            nxt_w, nxt_rel = {}, None
            wpf_sb, wcc_sb, wa_sb = cur_w["wpf"], cur_w["wcc"], cur_w["wa"]
            b3_sb, rel_sb = cur_w["b3"], cur_rel
            b1_sb = cur_w.get("b1")
            pad = PADS[i]

            if i != 0:
                xt_tiles, xt_next = xt_next, [None] * nT

            def emit_B(b):
                tt, off = b // 4, (b % 4) * 128
                ps = pu_ps.tile([128, 512], F32, tag="ps")
                nc.tensor.matmul(ps[:, :], xt_tiles[tt][:, 0, off:off + 128],
                                 wpf_sb[:, 0, :], start=True, stop=False)
                nc.tensor.matmul(ps[:, :], xt_tiles[tt][:, 1, off:off + 128],
                                 wpf_sb[:, 1, :], start=False, stop=True)
                st = stp.tile([128, 512], BF16, tag="st")
                st_tiles[b] = st
                nc.scalar.activation(st[:, :], ps[:, :], AF.Copy)

            def emit_S(b):
                S = sp.tile([128, 384], BF16, tag="S")
                S_tiles[b] = S
                nc.vector.tensor_scalar(S[:, :], rel_sb[:, b, :],
                                        iota_sb[:, 0:1], None, OP.is_equal)

            def emit_T(tt):
                pcs = _pieces(tt, nR, T, pad)
                v = vp.tile([128, 2, 516], BF16, tag="v")
                v_tiles[tt] = v
                for ob in range(2):
                    pw = pw_ps.tile([128, 512], F32, tag="ps")
                    nc.tensor.matmul(pw[:, :], wcc_sb[:, 0, ob, :],
                                     xt_tiles[tt][:, 0, :],
                                     start=True, stop=False)
                    for (j, ra, rb, pa) in pcs:
                        half = 0 if ra < 192 else 256
                        nc.tensor.matmul(
                            pw[:, pa:pa + rb - ra],
                            st_tiles[j][:, half + ob * 128:
                                        half + ob * 128 + 128],
                            S_tiles[j][:, ra:rb], start=False, stop=False)
                    nc.tensor.matmul(pw[:, :], wcc_sb[:, 1, ob, :],
                                     xt_tiles[tt][:, 1, :],
                                     start=False, stop=True)
                    if has_b1:
                        nc.scalar.activation(v[:, ob, 1:513], pw[:, :],
                                             AF.Prelu, alpha=SLOPE,
                                             bias=b1_sb[:, ob:ob + 1])
                    else:
                        nc.scalar.activation(v[:, ob, 1:513], pw[:, :],
                                             AF.Prelu, alpha=SLOPE)
                if tt == 0:
                    nc.vector.memset(v[:, :, 0:1], 0.0)
                else:
                    nc.vector.tensor_copy(v[:, :, 0:1],
                                          v_tiles[tt - 1][:, :, 512:513])
                    nc.vector.tensor_copy(v_tiles[tt - 1][:, :, 513:514],
                                          v[:, :, 1:2])
                    emit_conv3(tt - 1, wa_sb, b3_sb, i == NITER - 1)
                    if i + 1 < NITER:
                        xt_next[tt - 1] = emit_A(tt - 1)

            for b in range(nR + LAG):
                # consume (T) before produce (B): a B block waiting on the
                # x stream must not head-block a ready T tile on the PE
                if b >= LAG and (b - LAG) % 4 == 0:
                    emit_T((b - LAG) // 4)
                if b < nR:
                    if i == 0:
                        if b < 4:
                            # blockwise lrelu of tile 0 keeps the first
                            # B matmuls fed during the initial x stream
                            if b == 0:
                                xt0 = xtp.tile([128, 2, 512], BF16,
                                               tag="xt")
                                xt_tiles[0] = xt0
                            emit_A0_block(b)
                        else:
                            # lazy lrelu with one-tile lookahead
                            for t2 in (b // 4, b // 4 + 1):
                                if t2 < nT and xt_tiles[t2] is None:
                                    xt_tiles[t2] = emit_A(t2)
                    emit_B(b)
                    # iteration 0 defers the first S-builds until the
                    # tile-0/1 lrelus are emitted, so a late rel chunk 0
                    # cannot head-block the B-stage warmup on the DVE
                    if i != 0 or b > 4:
                        emit_S(b)
                    elif b == 4:
                        for bb in range(5):
                            emit_S(bb)
                    if i + 1 < NITER:
                        # prefetch next iteration's weights + rel on the
                        # (by now idle) sync queue, once the x stream and
                        # iteration-0 rel chunks have fully dispatched
                        if b == 36:
                            nxt_w["wpf"] = load_w(i + 1, "wpf")
                        elif b == 37:
                            nxt_w["wcc"] = load_w(i + 1, "wcc")
                        elif b == 38:
                            nxt_w["wa"] = load_w(i + 1, "wa")
                        elif b == 39:
                            nxt_w["b3"] = load_w(i + 1, "b3")
                            if has_b1:
                                nxt_w["b1"] = load_w(i + 1, "b1")
                        elif b == 40:
                            nxt_rel = relp.tile([128, nR, 384],
                                                mybir.dt.int8, tag="rel")
                            load_rel_blocks(nxt_rel, i + 1, 0, nR)
            nc.vector.memset(v_tiles[nT - 1][:, :, 513:514], 0.0)
            emit_conv3(nT - 1, wa_sb, b3_sb, i == NITER - 1)
            if i + 1 < NITER:
                xt_next[nT - 1] = emit_A(nT - 1)

    nc.compile()
    return nc


def _to_bf16(a):
    return np.asarray(a, dtype=np.float32).astype(ml_dtypes.bfloat16)


def prep_in_maps(x, d, WC, bC, WP, bP, WF, bF, WA, bA, T=T_FULL):
    """Build the 8 per-core input maps from the full-problem arrays.
    Returns (in_maps, has_b1)."""
    x = np.asarray(x, dtype=np.float32)
    d = np.asarray(d, dtype=np.float32)
    WC, WP, WF, WA = (np.asarray(w, dtype=np.float32) for w in (WC, WP, WF, WA))
    bC, bP, bF, bA = (np.asarray(b, dtype=np.float32) for b in (bC, bP, bF, bA))
    nb = x.shape[0]
    nR = T // 128

    # weights stored in DRAM in the exact SBUF layout (partition dim first)
    wpf = np.empty((NITER, 128, 2, 512), np.float32)
    wcc = np.empty((NITER, 128, 2, 2, 128), np.float32)
    wa = np.empty((NITER, 128, 3, 2, 2, 128), np.float32)
    for i in range(NITER):
        wpfT = np.concatenate([WP[i].T, WF[i].T], axis=1)  # [c', 512]
        wpf[i] = wpfT.reshape(2, 128, 512).transpose(1, 0, 2)
        for cb in range(2):
            for ob in range(2):
                wcc[i, :, cb, ob] = \
                    WC[i][ob * 128:(ob + 1) * 128,
                          cb * 128:(cb + 1) * 128].T
        for k in range(3):
            waT = WA[i, :, :, k].T                         # [c', o]
            wa[i, :, k] = waT.reshape(2, 128, 2, 128) \
                .transpose(1, 0, 2, 3)
    b1 = (bC + bP + bF).astype(np.float32)                  # [NITER, 256]
    has_b1 = bool(np.any(b1 != 0))
    b3 = bA.reshape(NITER, 2, 128).transpose(0, 2, 1).copy()

    wpf, wcc, wa = _to_bf16(wpf), _to_bf16(wcc), _to_bf16(wa)
    iota = np.arange(128, dtype=np.float32).reshape(128, 1)

    tf = np.arange(T, dtype=np.float32)
    in_maps = []
    for b in range(nb):
        dv = d[b, 0].astype(np.float32)
        rel = np.full((NITER, nR, 384), -128, np.int8)
        for i, dil in enumerate(DILATIONS):
            dd = dv * np.float32(dil)
            idxP = np.round(tf - dd).astype(np.int64)
            idxF = np.round(tf + dd).astype(np.int64)
            for j in range(nR):
                # P window: t in [128j, 128j+192)
                a, e = 128 * j, min(128 * j + 192, T)
                hit = idxP[a:e] // 128 == j
                rel[i, j, 0:e - a] = np.where(
                    hit, idxP[a:e] - 128 * j, -128).astype(np.int8)
                # F window: t in [128j-64, 128j+128)
                w0 = 128 * j - 64
                a, e = max(0, w0), min(128 * j + 128, T)
                hit = idxF[a:e] // 128 == j
                rel[i, j, 192 + a - w0:192 + e - w0] = np.where(
                    hit, idxF[a:e] - 128 * j, -128).astype(np.int8)
        m = {
            "x": _to_bf16(x[b].reshape(2, 128, T)),
            "wpf": wpf, "wcc": wcc, "wa": wa, "b3": b3,
            "rel": np.broadcast_to(rel[:, None], (NITER, 128, nR, 384)).copy(),
            "iota": iota,
        }
        if has_b1:
            m["b1"] = b1.reshape(NITER, 2, 128).transpose(0, 2, 1).copy()
        in_maps.append(m)
    return in_maps, has_b1


_nc_cache = {}


def kernel(**inputs) -> np.ndarray:
    T = inputs["x"].shape[2]
    in_maps, has_b1 = prep_in_maps(**inputs, T=T)
    key = (T, has_b1)
    if key not in _nc_cache:
        _nc_cache[key] = build_nc(T, has_b1=has_b1)
    nc = _nc_cache[key]
    res = run_bass_kernel_spmd(nc, in_maps, core_ids=list(range(8)))
    out = np.stack([np.asarray(res.results[i]["out"], dtype=np.float32)
                    .reshape(C, T) for i in range(8)])
    return out
